# revision 22
# baseline (speedup 1.0000x reference)
"""Multi-head graph attention message passing on 8 Trainium2 cores.

Strategy (dst-sharded, one SWDGE gather per edge):
  - Nodes sharded by dst across 8 cores (12500 each).
  - Per core, edges split into 4 src-windows of 25600 nodes so gather
    indices fit int16. Per window, own dsts are sorted by window-degree
    and grouped into blocks of 128; a dst's rank%128 picks its SBUF
    partition, so:
      * Q[dst] is a per-partition broadcast from an SBUF-resident Q table
        (no per-edge Q gather), and
      * segment_sum is a DVE reduce over the free dim (no scatter-add).
  - The only per-edge SWDGE op is the K|V row gather (bf16, 512B rows),
    spread round-robin over 4 SWDGE queues so Q7 descriptor generation
    parallelizes across core pairs.
  - Per-window K|V tables (separate DRAM tensors) let window w>0
    projections overlap window w-1 edge compute. Projection bias is a
    k=1 ones x bias matmul accumulated in PSUM; the PSUM->SBUF cast runs
    on the Scalar engine. Scores/wv accumulate fp32, I/O mostly bf16;
    host sums window partials and unpermutes.
"""

import numpy as np
import ml_dtypes

import concourse.bacc as bacc
import concourse.mybir as mybir
import concourse.tile as tile
from concourse.bass_utils import run_bass_kernel_spmd

F32 = mybir.dt.float32
BF16 = mybir.dt.bfloat16
I16 = mybir.dt.int16


class Cfg:
    n_nodes = 100000
    n_edges = 1600000
    in_dim = 128
    heads = 8
    hdim = 16
    hid = 128
    n_cores = 8
    own = 12500
    own_pad = 12544          # 98 blocks of 128
    n_blocks = 98
    nw = 4                   # src windows
    win = 25600              # nodes per window
    win_pad = 26112          # 51 * 512, window rows in its kv table
    pad_idx = 25600          # window-relative zero pad row index
    seg_cols = 32            # target cols per gathered segment tile
    gat_cols = 8             # max cols per dma_gather call (1024 idxs)
    op_cols = 32             # max cols per DVE op group
    proj_tile = 512

    def __init__(self, **kw):
        for k, v in kw.items():
            setattr(self, k, v)
        self.qn_pad = 12800   # own_pad padded to proj_tile multiple


def make_plan(cfg, widths):
    """widths: [nw][n_blocks] static col-widths (max over cores).
    Returns per-window segment/opgroup/gather structure + idx layout."""
    plan = []
    col = 0
    for w in range(cfg.nw):
        wl = widths[w]
        zb = cfg.n_blocks
        for b in range(cfg.n_blocks):
            if wl[b] == 0:
                zb = b
                break
        segs = []
        b = 0
        while b < zb:
            b0, cols = b, 0
            while b < zb and (cols == 0 or cols + wl[b] <= cfg.seg_cols):
                cols += wl[b]
                b += 1
            # op groups: runs of equal W, each <= op_cols, k <= 8
            ops = []
            oc = 0
            bb = b0
            while bb < b:
                W = wl[bb]
                k = 1
                while (bb + k < b and wl[bb + k] == W and k < 8
                       and (k + 1) * W <= cfg.op_cols):
                    k += 1
                ops.append((bb, k, W, oc))
                oc += k * W
                bb += k
            # gather calls: split cols into chunks <= gat_cols
            gats = []
            gc = 0
            while gc < cols:
                n = min(cfg.gat_cols, cols - gc)
                gats.append((gc, n))
                gc += n
            segs.append(dict(w=w, b0=b0, nb=b - b0, cols=cols,
                             col0=col, ops=ops, gats=gats))
            col += cols
        plan.append(dict(w=w, zb=zb, segs=segs))
    return plan, col  # col == total cols


def build_program(cfg, widths, need_clamp=True):
    plan, tot_cols = make_plan(cfg, widths)
    tot_idx = 128 * tot_cols
    segc = max(s["cols"] for pw in plan for s in pw["segs"])
    opc = max(k * W for pw in plan for s in pw["segs"]
              for (_, k, W, _) in s["ops"])
    opk = max(k for pw in plan for s in pw["segs"]
              for (_, k, W, _) in s["ops"])
    opg = max(len(s["ops"]) for pw in plan for s in pw["segs"])

    nc = bacc.Bacc("TRN2", target_bir_lowering=False, debug=False,
                   num_swdge_queues=4)

    xt = nc.dram_tensor("xt", [cfg.in_dim, cfg.nw * cfg.win_pad], BF16,
                        kind="ExternalInput")
    xt_own = nc.dram_tensor("xt_own", [cfg.nw, cfg.in_dim, cfg.qn_pad],
                            BF16, kind="ExternalInput")
    w_kv = nc.dram_tensor("w_kv", [cfg.in_dim, 2 * cfg.hid], BF16,
                          kind="ExternalInput")
    b_kv4 = nc.dram_tensor("b_kv4", [1, 4, 2 * cfg.hid], BF16,
                           kind="ExternalInput")
    w_q = nc.dram_tensor("w_q", [cfg.in_dim, cfg.hid], BF16,
                         kind="ExternalInput")
    b_q4 = nc.dram_tensor("b_q4", [1, 4, cfg.hid], BF16,
                          kind="ExternalInput")
    idx = nc.dram_tensor("idx", [128, tot_idx // 16], I16,
                         kind="ExternalInput")
    wv = nc.dram_tensor("wv", [cfg.nw * cfg.own_pad, cfg.hid], BF16,
                        kind="ExternalOutput")

    kv_tabs = [nc.dram_tensor(f"kv_tab{w}", [cfg.win_pad, 2 * cfg.hid],
                              BF16) for w in range(cfg.nw)]

    PT = cfg.proj_tile
    lim = 5.0 * float(np.sqrt(cfg.hdim))
    gq = [0]  # gather queue round robin

    with tile.TileContext(nc) as tc:
        with (
            tc.tile_pool(name="const", bufs=1) as cpool,
            tc.tile_pool(name="proj", bufs=3) as ppool,
            tc.tile_pool(name="psum", bufs=2, space="PSUM") as psum,
            tc.tile_pool(name="qsum", bufs=2, space="PSUM") as qsum,
            tc.tile_pool(name="qtab", bufs=2) as qpool,
            tc.tile_pool(name="seg", bufs=3) as spool,
            tc.tile_pool(name="edge", bufs=3) as epool,
            tc.tile_pool(name="etail", bufs=2) as tpool,
            tc.tile_pool(name="expool", bufs=opg + 2) as expool,
        ):
            w_kv_t = cpool.tile([cfg.in_dim, 2 * cfg.hid], BF16)
            b_kv_t = cpool.tile([1, 4, 2 * cfg.hid], BF16)
            w_q_t = cpool.tile([cfg.in_dim, cfg.hid], BF16)
            b_q_t = cpool.tile([1, 4, cfg.hid], BF16)
            nc.sync.dma_start(w_kv_t[:], w_kv[:])
            nc.sync.dma_start(b_kv_t[:], b_kv4[:])
            nc.sync.dma_start(w_q_t[:], w_q[:])
            nc.sync.dma_start(b_q_t[:], b_q4[:])
            ones_t = cpool.tile([1, 128], BF16)
            nc.vector.memset(ones_t[:], 1.0)
            zt = cpool.tile([128, cfg.hid], BF16)
            nc.vector.memset(zt[:], 0.0)
            idx_t = cpool.tile([128, tot_idx // 16], I16)
            nc.sync.dma_start(idx_t[:], idx[:])

            def proj_kv(w):
                """Project window w's nodes into kv_tabs[w] (bf16).
                Generator: yields after each group so the caller can
                interleave emission with edge segments (avoids Scalar
                FIFO head-of-line blocking)."""
                for g in range(cfg.win_pad // PT):
                    xt_t = ppool.tile([128, PT], BF16, tag="xt_t")
                    nc.sync.dma_start(
                        xt_t[:],
                        xt[:, w * cfg.win_pad + g * PT:
                           w * cfg.win_pad + (g + 1) * PT])
                    ps = psum.tile([128, PT // 128, 2 * cfg.hid], F32)
                    for h in range(2):
                        nc.tensor.matmul(
                            ps[:, 2 * h:2 * h + 2, :].rearrange(
                                "p s e -> p (s e)"),
                            ones_t[:],
                            b_kv_t[:, 2 * h:2 * h + 2, :].rearrange(
                                "p s e -> p (s e)"),
                            start=True, stop=False)
                    for s in range(PT // 128):
                        nc.tensor.matmul(
                            ps[:, s, :], xt_t[:, s * 128:(s + 1) * 128],
                            w_kv_t[:], start=False, stop=True)
                    out_sb = ppool.tile([128, PT // 128, 2 * cfg.hid],
                                        BF16, tag="out_sb")
                    nc.scalar.copy(out_sb[:], ps[:])
                    nc.sync.dma_start(
                        kv_tabs[w][g * PT:(g + 1) * PT, :].rearrange(
                            "(s p) e -> p s e", p=128),
                        out_sb[:])
                    yield
                # zero the V half of the pad row
                nc.sync.dma_start(
                    kv_tabs[w][cfg.pad_idx:cfg.pad_idx + 1, cfg.hid:],
                    zt[:1, :])

            def proj_q(w, q_sb):
                """Project window w's permuted own nodes into q_sb.
                Generator; yields after each group."""
                for g in range(cfg.qn_pad // PT):
                    xo_t = ppool.tile([128, PT], BF16, tag="xo_t")
                    nc.sync.dma_start(
                        xo_t[:], xt_own[w, :, g * PT:(g + 1) * PT])
                    qs = qsum.tile([128, PT // 128, cfg.hid], F32)
                    nc.tensor.matmul(
                        qs[:].rearrange("p s e -> p (s e)"),
                        ones_t[:],
                        b_q_t[:].rearrange("p s e -> p (s e)"),
                        start=True, stop=False)
                    for s in range(PT // 128):
                        nc.tensor.matmul(
                            qs[:, s, :], xo_t[:, s * 128:(s + 1) * 128],
                            w_q_t[:], start=False, stop=True)
                    nc.scalar.copy(
                        q_sb[:, g * (PT // 128):(g + 1) * (PT // 128), :],
                        qs[:])
                    yield

            def drain(gen):
                for _ in gen:
                    pass

            def qtile():
                q_sb = qpool.tile([128, cfg.qn_pad // 128, cfg.hid], BF16,
                                  tag="q_sb")
                return q_sb

            q_sbs = {w: None for w in range(cfg.nw)}
            drain(proj_kv(0))
            q_sbs[0] = qtile()
            drain(proj_q(0, q_sbs[0]))
            q_sbs[1] = qtile()
            drain(proj_q(1, q_sbs[1]))

            for w in range(cfg.nw):
                q_sb = q_sbs[w]
                # feeders: next window's kv proj + window-after-next's q proj,
                # interleaved between this window's segments
                feeders = []
                if w + 1 < cfg.nw:
                    feeders.append(proj_kv(w + 1))
                if w + 2 < cfg.nw:
                    q_sbs[w + 2] = qtile()
                    feeders.append(proj_q(w + 2, q_sbs[w + 2]))
                segs = plan[w]["segs"]
                nfeed = (cfg.win_pad + cfg.qn_pad) // PT
                per_seg = -(-nfeed // max(1, len(segs))) + 1
                for seg in segs:
                    kv_t = spool.tile([128, segc, 2 * cfg.hid], BF16,
                                      tag="kv_t")
                    for (gc, ncol) in seg["gats"]:
                        n = 128 * ncol
                        o = (seg["col0"] + gc) * 8  # 128/16 per col
                        nc.gpsimd.dma_gather(
                            kv_t[:, gc:gc + ncol, :], kv_tabs[w][:],
                            idx_t[:, o:o + ncol * 8], n, n, 2 * cfg.hid,
                            queue_num=gq[0] % 4, single_packet=True)
                        gq[0] += 1

                    # pass 1: score chains (keeps DVE queue free of
                    # exp-dependent ops -> no head-of-line blocking)
                    exs = []
                    for (b0, k, W, oc) in seg["ops"]:
                        kW = k * W
                        kview = kv_t[:, oc:oc + kW, :cfg.hid]
                        prod = epool.tile([128, opc, cfg.hid], BF16,
                                          tag="prod")
                        nc.vector.tensor_mul(
                            prod[:, :kW, :].rearrange(
                                "p (k u) f -> p k u f", k=k),
                            kview.rearrange("p (k u) f -> p k u f", k=k),
                            q_sb[:, b0:b0 + k, :].unsqueeze(2).broadcast_to(
                                [128, k, W, cfg.hid]))
                        sc = epool.tile([128, opc, cfg.heads], F32,
                                        tag="sc")
                        nc.vector.reduce_sum(
                            sc[:, :kW, :],
                            prod[:, :kW, :].rearrange(
                                "p c (h d) -> p c h d", d=cfg.hdim),
                            axis=mybir.AxisListType.X)
                        if need_clamp:
                            nc.vector.tensor_scalar(
                                sc[:, :kW, :], sc[:, :kW, :], lim, -lim,
                                mybir.AluOpType.min, mybir.AluOpType.max)
                        ex = expool.tile([128, opc, cfg.heads], BF16,
                                         tag="ex")
                        nc.scalar.activation(
                            ex[:, :kW, :], sc[:, :kW, :],
                            mybir.ActivationFunctionType.Exp,
                            scale=float(1.0 / np.sqrt(cfg.hdim)))
                        exs.append(ex)

                    # pass 2: message + segment-sum chains
                    for ex, (b0, k, W, oc) in zip(exs, seg["ops"]):
                        kW = k * W
                        vview = kv_t[:, oc:oc + kW, cfg.hid:]
                        msg = epool.tile([128, opc, cfg.hid], BF16,
                                         tag="msg")
                        nc.vector.tensor_mul(
                            msg[:, :kW, :].rearrange(
                                "p c (h d) -> p c h d", d=cfg.hdim),
                            vview.rearrange(
                                "p c (h d) -> p c h d", d=cfg.hdim),
                            ex[:, :kW, :].unsqueeze(-1).broadcast_to(
                                [128, kW, cfg.heads, cfg.hdim]))
                        wvb = tpool.tile([128, opk, cfg.hid], BF16,
                                         tag="wvb")
                        with nc.allow_low_precision(
                                reason="DVE reduce accumulates fp32; "
                                       "only the final store is bf16"):
                            nc.vector.reduce_sum(
                                wvb[:, :k, :],
                                msg[:, :kW, :].rearrange(
                                    "p (k u) f -> p k f u", k=k),
                                axis=mybir.AxisListType.X)
                        r0 = w * cfg.own_pad + 128 * b0
                        nc.scalar.dma_start(
                            wv[r0:r0 + 128 * k, :].rearrange(
                                "(s p) e -> p s e", p=128),
                            wvb[:, :k, :])
                    for _ in range(per_seg):
                        while feeders:
                            try:
                                next(feeders[0])
                                break
                            except StopIteration:
                                feeders.pop(0)
                for f in feeders:
                    drain(f)
    nc.finalize()
    return nc


def _wrap16(a):
    n = len(a)
    w = a.reshape(n // 16, 16).T.astype(np.int16)
    return np.tile(w, (8, 1))


def _cumcount(sorted_vals):
    n = len(sorted_vals)
    if n == 0:
        return np.empty(0, np.int64)
    flag = np.empty(n, bool)
    flag[0] = True
    flag[1:] = sorted_vals[1:] != sorted_vals[:-1]
    starts = np.flatnonzero(flag)
    reps = np.diff(np.append(starts, n))
    return np.arange(n) - np.repeat(starts, reps)


def prepare_inputs(cfg, x, src, dst, Wq, bq, Wk, bk, Wv, bv):
    bf = ml_dtypes.bfloat16
    x = np.asarray(x, np.float32)
    src = np.asarray(src, np.int64)
    dst = np.asarray(dst, np.int64)

    xt = np.zeros((cfg.in_dim, cfg.nw * cfg.win_pad), bf)
    for w in range(cfg.nw):
        n0 = w * cfg.win
        n1 = min(cfg.n_nodes, n0 + cfg.win)
        xt[:, w * cfg.win_pad:w * cfg.win_pad + (n1 - n0)] = \
            x[n0:n1].T.astype(bf)

    w_kv = np.concatenate([np.asarray(Wk, np.float32),
                           np.asarray(Wv, np.float32)], axis=1).astype(bf)
    b_kv4 = np.tile(np.concatenate(
        [np.asarray(bk, np.float32), np.asarray(bv, np.float32)])[None, :],
        (4, 1))[None].astype(bf)
    w_q = np.asarray(Wq, np.float32).astype(bf)
    b_q4 = np.tile(np.asarray(bq, np.float32)[None, :], (4, 1))[None].astype(bf)

    core_of = dst // cfg.own
    win_of = src // cfg.win

    percore = []
    for c in range(cfg.n_cores):
        in_c = np.nonzero(core_of == c)[0]
        s_c, d_c = src[in_c], dst[in_c] - c * cfg.own
        w_c = win_of[in_c]
        wins = []
        for w in range(cfg.nw):
            m = w_c == w
            s_w, d_w = s_c[m], d_c[m]
            deg = np.bincount(d_w, minlength=cfg.own_pad)
            order = np.argsort(-deg, kind="stable")
            rank = np.empty(cfg.own_pad, np.int64)
            rank[order] = np.arange(cfg.own_pad)
            wins.append((s_w, d_w, deg, order, rank))
        percore.append(wins)

    # static width table: max over cores of block-leading degree
    widths = []
    for w in range(cfg.nw):
        wl = np.zeros(cfg.n_blocks, np.int64)
        for c in range(cfg.n_cores):
            deg, order = percore[c][w][2], percore[c][w][3]
            wl = np.maximum(wl, deg[order[::128][:cfg.n_blocks]])
        widths.append(wl.tolist())

    plan, tot_cols = make_plan(cfg, widths)
    tot_idx = 128 * tot_cols

    # per-window global col start of each block
    colstart = np.zeros((cfg.nw, cfg.n_blocks), np.int64)
    for w in range(cfg.nw):
        for seg in plan[w]["segs"]:
            cc = seg["col0"]
            for b in range(seg["b0"], seg["b0"] + seg["nb"]):
                colstart[w][b] = cc
                cc += widths[w][b]

    in_maps = []
    orders = []
    for c in range(cfg.n_cores):
        idx_all = np.full(tot_idx, cfg.pad_idx, np.int64)
        xo = np.zeros((cfg.nw, cfg.in_dim, cfg.qn_pad), bf)
        ords = []
        for w in range(cfg.nw):
            s_w, d_w, deg, order, rank = percore[c][w]
            ords.append(order)
            r = rank[d_w]
            o2 = np.argsort(r, kind="stable")
            rs = r[o2]
            cc = _cumcount(rs)
            p = rs % 128
            b = rs // 128
            col = colstart[w][b] + cc
            pos = col * 128 + p
            idx_all[pos] = s_w[o2] - w * cfg.win
            valid = order < cfg.own
            xsel = np.zeros((cfg.own_pad, cfg.in_dim), np.float32)
            xsel[valid] = x[c * cfg.own + order[valid]]
            xo[w, :, :cfg.own_pad] = xsel.T.astype(bf)
        in_maps.append({
            "xt": xt, "xt_own": xo, "w_kv": w_kv, "b_kv4": b_kv4,
            "w_q": w_q, "b_q4": b_q4, "idx": _wrap16(idx_all),
        })
        orders.append(ords)

    # Does the score clip at +-5 ever fire? If the true max is safely
    # below, the on-device clamp ops are dead weight and are omitted.
    Kf = x @ np.asarray(Wk, np.float32) + np.asarray(bk, np.float32)
    Qf = x @ np.asarray(Wq, np.float32) + np.asarray(bq, np.float32)
    Kf = Kf.reshape(-1, cfg.heads, cfg.hdim)
    Qf = Qf.reshape(-1, cfg.heads, cfg.hdim)
    mx = 0.0
    scale = float(np.sqrt(cfg.hdim))
    for i in range(0, len(src), 200000):
        s, d = src[i:i + 200000], dst[i:i + 200000]
        scs = np.einsum("ehd,ehd->eh", Kf[s], Qf[d]) / scale
        mx = max(mx, float(np.abs(scs).max()))
    need_clamp = mx > 4.5
    return in_maps, widths, plan, orders, need_clamp


def assemble(cfg, plan, orders, results):
    out = np.zeros((cfg.n_nodes, cfg.hid), np.float32)
    for c in range(cfg.n_cores):
        wv = results[c]["wv"].astype(np.float32)
        for w in range(cfg.nw):
            zb = plan[w]["zb"]
            nrow = 128 * zb
            h = wv[w * cfg.own_pad: w * cfg.own_pad + nrow]
            order = orders[c][w][:nrow]
            valid = order < cfg.own
            out[c * cfg.own + order[valid]] += h[valid]
    return out.reshape(cfg.n_nodes, cfg.heads, cfg.hdim)


def run(inputs, trace=False):
    cfg = Cfg()
    in_maps, widths, plan, orders, need_clamp = prepare_inputs(cfg, **inputs)
    nc = build_program(cfg, widths, need_clamp=need_clamp)
    res = run_bass_kernel_spmd(nc, in_maps, list(range(cfg.n_cores)),
                               trace=trace)
    return assemble(cfg, plan, orders, res.results), res


def kernel(x, src, dst, Wq, bq, Wk, bk, Wv, bv):
    out, _ = run(dict(x=x, src=src, dst=dst, Wq=Wq, bq=bq,
                      Wk=Wk, bk=bk, Wv=Wv, bv=bv))
    return out
